# revision 8
# baseline (speedup 1.0000x reference)
"""Trainium2 Bass kernel for nn_Basemask (sparse_attention, memory-bound).

Reference computes, from ragged per-graph node features x [N,F] and a sorted
PyG batch vector:
  dense_x  [B, Nmax, F]       — scatter of x rows into per-graph padded slabs
  Dmask    [B, Nmax] bool     — valid-node mask
  attn_mask[B, H, Nmax, Nmax] — key-padding rows (0 / -1e9) broadcast over H, Q

Sharding: by graph dim B across 8 cores (8 graphs each). batch_ids is sorted,
so each core's node rows are one contiguous slice of x. Host computes only the
tiny bincount/offset metadata (needed for the slicing anyway); all heavy data
movement (scatter of x -> dense_x, materialization of the 2.25 GiB attn_mask)
happens on device.
"""

import sys

import numpy as np

sys.path.insert(0, "/opt/trn_rl_repo")

# Problem shapes (hardcoded per spec)
N_TOTAL = 32768
F = 512
B = 64
NMAX = 768
H = 16
NCORES = 8
GPC = B // NCORES           # graphs per core = 8
DENSE = GPC * NMAX          # dense rows per core = 6144
ATT_ROWS = GPC * H * NMAX   # attn rows per core = 98304
XS_ROWS = DENSE + 128       # per-core input slab, padded; last rows stay zero
NEG_INF = -1000000000.0

REPS = 4                    # mask-row repeats per SBUF mask tile
MTILE = REPS * NMAX         # 3072 f32 = 12 KiB / partition
ATT_CHUNK_ROWS = 128 * REPS                 # output rows per attn DMA (512)
ATT_CHUNKS = H * NMAX // ATT_CHUNK_ROWS     # attn DMAs per graph (24)
W_GROUPS = DENSE // 512     # dense write groups (12), 1 MiB each
META_GIDX0 = 16             # first gidx column inside the packed meta input


def build_nc():
    import concourse.bacc as bacc
    import concourse.bass as bass
    import concourse.mybir as mybir
    import concourse.tile as tile

    dt = mybir.dt
    nc = bacc.Bacc(None, target_bir_lowering=False)

    xs = nc.dram_tensor("xs", [XS_ROWS, F], dt.float32, kind="ExternalInput")
    meta = nc.dram_tensor("meta", [128, 64], dt.float32, kind="ExternalInput")

    yd = nc.dram_tensor("yd", [DENSE, F], dt.float32, kind="ExternalOutput")
    ym = nc.dram_tensor("ym", [GPC, NMAX], dt.uint8, kind="ExternalOutput")
    ya = nc.dram_tensor("ya", [ATT_ROWS, NMAX], dt.float32, kind="ExternalOutput")

    with tile.TileContext(nc) as tc:
        with (
            tc.tile_pool(name="const", bufs=1) as cpool,
            tc.tile_pool(name="xt", bufs=3) as xpool,
        ):
            meta_sb = cpool.tile([128, 64], dt.float32, tag="meta")
            nc.sync.dma_start(meta_sb[:], meta[:])

            # iota over the free dim, value = j % NMAX, same on all partitions.
            # f32 is exact here (values < 2^24).
            itf = cpool.tile([128, MTILE], dt.float32, tag="itf")
            nc.gpsimd.iota(
                itf[:],
                pattern=[[0, REPS], [1, NMAX]],
                base=0,
                channel_multiplier=0,
                allow_small_or_imprecise_dtypes=True,
            )

            # Dmask: partition g holds graph g's row; (j < count_g) -> 1/0 u8
            dm = cpool.tile([GPC, NMAX], dt.uint8, tag="dm")
            nc.vector.tensor_scalar(
                out=dm[:],
                in0=itf[0:GPC, 0:NMAX],
                scalar1=meta_sb[0:GPC, 8:9],
                scalar2=None,
                op0=mybir.AluOpType.is_lt,
            )
            nc.sync.dma_start(ym[:], dm[:])

            # attn_mask: per graph, one GPSIMD op builds the repeated key-pad
            # row tile [128, 4*768] (identical on all partitions); 24 DMAs
            # stream it to the 12288 output rows (1.5 MiB contiguous each).
            # One dedicated tile per graph -> no WAR reuse waits.
            for g in range(GPC):
                m = cpool.tile([128, MTILE], dt.float32, tag=f"m{g}")
                nc.vector.tensor_scalar(
                    out=m[:],
                    in0=itf[:],
                    scalar1=meta_sb[:, g : g + 1],
                    scalar2=NEG_INF,
                    op0=mybir.AluOpType.is_ge,
                    op1=mybir.AluOpType.mult,
                )
                for c in range(ATT_CHUNKS):
                    r0 = g * H * NMAX + c * ATT_CHUNK_ROWS
                    nc.sync.dma_start(ya[r0 : r0 + ATT_CHUNK_ROWS, :], m[:])

            # dense_x: the HW gather consumes ONE index per partition and
            # reads out_free_size contiguous elements from it. Exploit that:
            # per graph, idx[p] = rel_g + 6p gathers 6 contiguous rows
            # (one 12 KiB descriptor per partition -> near line-rate), then a
            # fused DVE op zeroes the padding rows in place:
            #   xg = (row_idx < count_g) * xg,  row_idx(p, j) = 6p + j//F
            # and one 1.5 MiB write stores graph g's slab.
            rowiota = cpool.tile([128, 6 * F], dt.float32, tag="rowiota")
            nc.gpsimd.iota(
                rowiota[:],
                pattern=[[1, 6], [0, F]],
                base=0,
                channel_multiplier=6,
                allow_small_or_imprecise_dtypes=True,
            )
            for g in range(GPC):
                xg = xpool.tile([128, 6 * F], dt.float32)
                idx_ap = meta_sb[:, META_GIDX0 + g : META_GIDX0 + g + 1]
                nc.gpsimd.indirect_dma_start(
                    out=xg[:],
                    out_offset=None,
                    in_=xs[:],
                    in_offset=bass.IndirectOffsetOnAxis(
                        ap=idx_ap.bitcast(dt.int32), axis=0
                    ),
                )
                nc.vector.scalar_tensor_tensor(
                    out=xg[:],
                    in0=rowiota[:],
                    scalar=meta_sb[:, g : g + 1],
                    in1=xg[:],
                    op0=mybir.AluOpType.is_lt,
                    op1=mybir.AluOpType.mult,
                )
                nc.scalar.dma_start(yd[g * NMAX : (g + 1) * NMAX, :], xg[:])

    nc.compile()
    return nc


def make_in_maps(x, batch_ids):
    x = np.ascontiguousarray(np.asarray(x), dtype=np.float32)
    bids = np.asarray(batch_ids).astype(np.int64)
    counts = np.bincount(bids, minlength=B)[:B]
    starts = np.cumsum(counts) - counts

    in_maps = []
    p = np.arange(128)
    for k in range(NCORES):
        g0 = k * GPC
        r0 = int(starts[g0])
        r1 = int(starts[g0 + GPC - 1] + counts[g0 + GPC - 1])
        xs = np.zeros((XS_ROWS, F), np.float32)
        xs[: r1 - r0] = x[r0:r1]
        rel = (starts[g0 : g0 + GPC] - r0).astype(np.int64)
        cnt = counts[g0 : g0 + GPC].astype(np.int64)

        # per-graph gather base rows: partition p reads xs rows rel_g+6p..+5
        gidx = (rel[None, :] + 6 * p[:, None]).astype(np.int32)   # [128, GPC]

        meta = np.zeros((128, 64), np.float32)
        meta[:, :GPC] = cnt.astype(np.float32)           # replicated counts
        meta[:GPC, 8] = cnt.astype(np.float32)           # per-partition counts
        meta[:, META_GIDX0 : META_GIDX0 + GPC] = gidx.view(np.float32)

        in_maps.append({"xs": xs, "meta": meta})
    return in_maps


def assemble(results):
    dense = np.concatenate(
        [r["yd"].reshape(GPC, NMAX, F) for r in results], axis=0
    )
    dmask = np.concatenate([r["ym"] for r in results], axis=0).astype(bool)
    attn = np.concatenate(
        [r["ya"].reshape(GPC, H, NMAX, NMAX) for r in results], axis=0
    )
    return dense, dmask, attn


def kernel(x, batch_ids, batch_size, max_num_nodes, num_heads, **_):
    from concourse.bass_utils import run_bass_kernel_spmd

    in_maps = make_in_maps(x, batch_ids)
    nc = build_nc()
    res = run_bass_kernel_spmd(nc, in_maps, core_ids=list(range(NCORES)))
    return assemble(res.results)


# revision 9
# speedup vs baseline: 1.0523x; 1.0523x over previous
"""Trainium2 Bass kernel for nn_Basemask (sparse_attention, memory-bound).

Reference computes, from ragged per-graph node features x [N,F] and a sorted
PyG batch vector:
  dense_x  [B, Nmax, F]       — scatter of x rows into per-graph padded slabs
  Dmask    [B, Nmax] bool     — valid-node mask
  attn_mask[B, H, Nmax, Nmax] — key-padding rows (0 / -1e9) broadcast over H, Q

Sharding: by graph dim B across 8 cores (8 graphs each). batch_ids is sorted,
so each core's node rows are one contiguous slice of x. Host computes only the
tiny bincount/offset metadata (needed for the slicing anyway); all heavy data
movement (scatter of x -> dense_x, materialization of the 2.25 GiB attn_mask)
happens on device.
"""

import sys

import numpy as np

sys.path.insert(0, "/opt/trn_rl_repo")

# Problem shapes (hardcoded per spec)
N_TOTAL = 32768
F = 512
B = 64
NMAX = 768
H = 16
NCORES = 8
GPC = B // NCORES           # graphs per core = 8
DENSE = GPC * NMAX          # dense rows per core = 6144
ATT_ROWS = GPC * H * NMAX   # attn rows per core = 98304
XS_ROWS = DENSE + 128       # per-core input slab, padded; last rows stay zero
NEG_INF = -1000000000.0

REPS = 4                    # mask-row repeats per SBUF mask tile
MTILE = REPS * NMAX         # 3072 f32 = 12 KiB / partition
ATT_CHUNK_ROWS = 128 * REPS                 # output rows per attn DMA (512)
ATT_CHUNKS = H * NMAX // ATT_CHUNK_ROWS     # attn DMAs per graph (24)
W_GROUPS = DENSE // 512     # dense write groups (12), 1 MiB each
META_GIDX0 = 16             # first gidx column inside the packed meta input


def build_nc():
    import concourse.bacc as bacc
    import concourse.bass as bass
    import concourse.mybir as mybir
    import concourse.tile as tile

    dt = mybir.dt
    nc = bacc.Bacc(None, target_bir_lowering=False)

    xs = nc.dram_tensor("xs", [XS_ROWS, F], dt.float32, kind="ExternalInput")
    meta = nc.dram_tensor("meta", [128, 64], dt.float32, kind="ExternalInput")

    yd = nc.dram_tensor("yd", [DENSE, F], dt.float32, kind="ExternalOutput")
    ym = nc.dram_tensor("ym", [GPC, NMAX], dt.uint8, kind="ExternalOutput")
    ya = nc.dram_tensor("ya", [ATT_ROWS, NMAX], dt.float32, kind="ExternalOutput")

    with tile.TileContext(nc) as tc:
        with (
            tc.tile_pool(name="const", bufs=1) as cpool,
            tc.tile_pool(name="xt", bufs=3) as xpool,
        ):
            meta_sb = cpool.tile([128, 64], dt.float32, tag="meta")
            nc.sync.dma_start(meta_sb[:], meta[:])

            # iota over the free dim, value = j % NMAX, same on all partitions.
            # f32 is exact here (values < 2^24).
            itf = cpool.tile([128, MTILE], dt.float32, tag="itf")
            nc.gpsimd.iota(
                itf[:],
                pattern=[[0, REPS], [1, NMAX]],
                base=0,
                channel_multiplier=0,
                allow_small_or_imprecise_dtypes=True,
            )

            # Dmask: partition g holds graph g's row; (j < count_g) -> 1/0 u8
            dm = cpool.tile([GPC, NMAX], dt.uint8, tag="dm")
            nc.vector.tensor_scalar(
                out=dm[:],
                in0=itf[0:GPC, 0:NMAX],
                scalar1=meta_sb[0:GPC, 8:9],
                scalar2=None,
                op0=mybir.AluOpType.is_lt,
            )
            nc.sync.dma_start(ym[:], dm[:])

            # attn_mask: per graph, one GPSIMD op builds the repeated key-pad
            # row tile [128, 4*768] (identical on all partitions); 24 DMAs
            # stream it to the 12288 output rows (1.5 MiB contiguous each).
            # One dedicated tile per graph -> no WAR reuse waits.
            att_reps = H * NMAX * NMAX // (128 * MTILE)   # 24
            for g in range(GPC):
                m = cpool.tile([128, MTILE], dt.float32, tag=f"m{g}")
                nc.vector.tensor_scalar(
                    out=m[:],
                    in0=itf[:],
                    scalar1=meta_sb[:, g : g + 1],
                    scalar2=NEG_INF,
                    op0=mybir.AluOpType.is_ge,
                    op1=mybir.AluOpType.mult,
                )
                # One 36 MiB DMA per graph: step-0 broadcast source re-reads
                # the mask tile 24x (every output row receives the same
                # repeated key-pad row, so source ordering is irrelevant).
                src_ap = m[:].unsqueeze(1).broadcast_to([128, att_reps, MTILE])
                r0 = g * H * NMAX
                nc.sync.dma_start(ya[r0 : r0 + H * NMAX, :], src_ap)

            # dense_x: the HW gather consumes ONE index per partition and
            # reads out_free_size contiguous elements from it. Exploit that:
            # per graph, idx[p] = rel_g + 6p gathers 6 contiguous rows
            # (one 12 KiB descriptor per partition -> near line-rate), then a
            # fused DVE op zeroes the padding rows in place:
            #   xg = (row_idx < count_g) * xg,  row_idx(p, j) = 6p + j//F
            # and one 1.5 MiB write stores graph g's slab.
            rowiota = cpool.tile([128, 6 * F], dt.float32, tag="rowiota")
            nc.gpsimd.iota(
                rowiota[:],
                pattern=[[1, 6], [0, F]],
                base=0,
                channel_multiplier=6,
                allow_small_or_imprecise_dtypes=True,
            )
            for g in range(GPC):
                xg = xpool.tile([128, 6 * F], dt.float32)
                idx_ap = meta_sb[:, META_GIDX0 + g : META_GIDX0 + g + 1]
                nc.gpsimd.indirect_dma_start(
                    out=xg[:],
                    out_offset=None,
                    in_=xs[:],
                    in_offset=bass.IndirectOffsetOnAxis(
                        ap=idx_ap.bitcast(dt.int32), axis=0
                    ),
                )
                nc.vector.scalar_tensor_tensor(
                    out=xg[:],
                    in0=rowiota[:],
                    scalar=meta_sb[:, g : g + 1],
                    in1=xg[:],
                    op0=mybir.AluOpType.is_lt,
                    op1=mybir.AluOpType.mult,
                )
                nc.scalar.dma_start(yd[g * NMAX : (g + 1) * NMAX, :], xg[:])

    nc.compile()
    return nc


def make_in_maps(x, batch_ids):
    x = np.ascontiguousarray(np.asarray(x), dtype=np.float32)
    bids = np.asarray(batch_ids).astype(np.int64)
    counts = np.bincount(bids, minlength=B)[:B]
    starts = np.cumsum(counts) - counts

    in_maps = []
    p = np.arange(128)
    for k in range(NCORES):
        g0 = k * GPC
        r0 = int(starts[g0])
        r1 = int(starts[g0 + GPC - 1] + counts[g0 + GPC - 1])
        xs = np.zeros((XS_ROWS, F), np.float32)
        xs[: r1 - r0] = x[r0:r1]
        rel = (starts[g0 : g0 + GPC] - r0).astype(np.int64)
        cnt = counts[g0 : g0 + GPC].astype(np.int64)

        # per-graph gather base rows: partition p reads xs rows rel_g+6p..+5
        gidx = (rel[None, :] + 6 * p[:, None]).astype(np.int32)   # [128, GPC]

        meta = np.zeros((128, 64), np.float32)
        meta[:, :GPC] = cnt.astype(np.float32)           # replicated counts
        meta[:GPC, 8] = cnt.astype(np.float32)           # per-partition counts
        meta[:, META_GIDX0 : META_GIDX0 + GPC] = gidx.view(np.float32)

        in_maps.append({"xs": xs, "meta": meta})
    return in_maps


def assemble(results):
    dense = np.concatenate(
        [r["yd"].reshape(GPC, NMAX, F) for r in results], axis=0
    )
    dmask = np.concatenate([r["ym"] for r in results], axis=0).astype(bool)
    attn = np.concatenate(
        [r["ya"].reshape(GPC, H, NMAX, NMAX) for r in results], axis=0
    )
    return dense, dmask, attn


def kernel(x, batch_ids, batch_size, max_num_nodes, num_heads, **_):
    from concourse.bass_utils import run_bass_kernel_spmd

    in_maps = make_in_maps(x, batch_ids)
    nc = build_nc()
    res = run_bass_kernel_spmd(nc, in_maps, core_ids=list(range(NCORES)))
    return assemble(res.results)


# revision 10
# speedup vs baseline: 1.1125x; 1.0572x over previous
"""Trainium2 Bass kernel for nn_Basemask (sparse_attention, memory-bound).

Reference computes, from ragged per-graph node features x [N,F] and a sorted
PyG batch vector:
  dense_x  [B, Nmax, F]       — scatter of x rows into per-graph padded slabs
  Dmask    [B, Nmax] bool     — valid-node mask
  attn_mask[B, H, Nmax, Nmax] — key-padding rows (0 / -1e9) broadcast over H, Q

Sharding: by graph dim B across 8 cores (8 graphs each). batch_ids is sorted,
so each core's node rows are one contiguous slice of x. Host computes only the
tiny bincount/offset metadata (needed for the slicing anyway); all heavy data
movement (scatter of x -> dense_x, materialization of the 2.25 GiB attn_mask)
happens on device.
"""

import sys

import numpy as np

sys.path.insert(0, "/opt/trn_rl_repo")

# Problem shapes (hardcoded per spec)
N_TOTAL = 32768
F = 512
B = 64
NMAX = 768
H = 16
NCORES = 8
GPC = B // NCORES           # graphs per core = 8
DENSE = GPC * NMAX          # dense rows per core = 6144
ATT_ROWS = GPC * H * NMAX   # attn rows per core = 98304
XS_ROWS = DENSE + 128       # per-core input slab, padded; last rows stay zero
NEG_INF = -1000000000.0

REPS = 4                    # mask-row repeats per SBUF mask tile
MTILE = REPS * NMAX         # 3072 f32 = 12 KiB / partition
ATT_CHUNK_ROWS = 128 * REPS                 # output rows per attn DMA (512)
ATT_CHUNKS = H * NMAX // ATT_CHUNK_ROWS     # attn DMAs per graph (24)
W_GROUPS = DENSE // 512     # dense write groups (12), 1 MiB each
META_GIDX0 = 16             # first gidx column inside the packed meta input


def build_nc():
    import concourse.bacc as bacc
    import concourse.bass as bass
    import concourse.mybir as mybir
    import concourse.tile as tile

    dt = mybir.dt
    nc = bacc.Bacc(None, target_bir_lowering=False)

    xs = nc.dram_tensor("xs", [XS_ROWS, F], dt.float32, kind="ExternalInput")
    meta = nc.dram_tensor("meta", [128, 64], dt.float32, kind="ExternalInput")

    yd = nc.dram_tensor("yd", [DENSE, F], dt.float32, kind="ExternalOutput")
    ym = nc.dram_tensor("ym", [GPC, NMAX], dt.uint8, kind="ExternalOutput")
    ya = nc.dram_tensor("ya", [ATT_ROWS, NMAX], dt.float32, kind="ExternalOutput")

    with tile.TileContext(nc) as tc:
        with (
            tc.tile_pool(name="const", bufs=1) as cpool,
            tc.tile_pool(name="xt", bufs=3) as xpool,
        ):
            meta_sb = cpool.tile([128, 64], dt.float32, tag="meta")
            nc.sync.dma_start(meta_sb[:], meta[:])

            # iota over the free dim, value = j % NMAX, same on all partitions.
            # f32 is exact here (values < 2^24).
            itf = cpool.tile([128, MTILE], dt.float32, tag="itf")
            nc.gpsimd.iota(
                itf[:],
                pattern=[[0, REPS], [1, NMAX]],
                base=0,
                channel_multiplier=0,
                allow_small_or_imprecise_dtypes=True,
            )

            # Dmask: partition g holds graph g's row; (j < count_g) -> 1/0 u8
            dm = cpool.tile([GPC, NMAX], dt.uint8, tag="dm")
            nc.vector.tensor_scalar(
                out=dm[:],
                in0=itf[0:GPC, 0:NMAX],
                scalar1=meta_sb[0:GPC, 8:9],
                scalar2=None,
                op0=mybir.AluOpType.is_lt,
            )
            nc.sync.dma_start(ym[:], dm[:])

            # attn_mask: per graph, one GPSIMD op builds the repeated key-pad
            # row tile [128, 4*768] (identical on all partitions); 24 DMAs
            # stream it to the 12288 output rows (1.5 MiB contiguous each).
            # One dedicated tile per graph -> no WAR reuse waits.
            # dense_x: the HW gather consumes ONE index per partition and
            # reads out_free_size contiguous elements from it. Exploit that:
            # per graph, idx[p] = rel_g + 6p gathers 6 contiguous rows
            # (one 12 KiB descriptor per partition -> near line-rate), then a
            # fused DVE op zeroes the padding rows in place:
            #   xg = (row_idx < count_g) * xg,  row_idx(p, j) = 6p + j//F
            # and one 1.5 MiB write stores graph g's slab.
            rowiota = cpool.tile([128, 6 * F], dt.float32, tag="rowiota")
            nc.gpsimd.iota(
                rowiota[:],
                pattern=[[1, 6], [0, F]],
                base=0,
                channel_multiplier=6,
                allow_small_or_imprecise_dtypes=True,
            )
            for g in range(GPC):
                xg = xpool.tile([128, 6 * F], dt.float32)
                idx_ap = meta_sb[:, META_GIDX0 + g : META_GIDX0 + g + 1]
                nc.gpsimd.indirect_dma_start(
                    out=xg[:],
                    out_offset=None,
                    in_=xs[:],
                    in_offset=bass.IndirectOffsetOnAxis(
                        ap=idx_ap.bitcast(dt.int32), axis=0
                    ),
                )
                nc.vector.scalar_tensor_tensor(
                    out=xg[:],
                    in0=rowiota[:],
                    scalar=meta_sb[:, g : g + 1],
                    in1=xg[:],
                    op0=mybir.AluOpType.is_lt,
                    op1=mybir.AluOpType.mult,
                )
                nc.scalar.dma_start(yd[g * NMAX : (g + 1) * NMAX, :], xg[:])

            att_reps = H * NMAX * NMAX // (128 * MTILE)   # 24
            for g in range(GPC):
                m = cpool.tile([128, MTILE], dt.float32, tag=f"m{g}")
                nc.vector.tensor_scalar(
                    out=m[:],
                    in0=itf[:],
                    scalar1=meta_sb[:, g : g + 1],
                    scalar2=NEG_INF,
                    op0=mybir.AluOpType.is_ge,
                    op1=mybir.AluOpType.mult,
                )
                # One 36 MiB DMA per graph: step-0 broadcast source re-reads
                # the mask tile 24x (every output row receives the same
                # repeated key-pad row, so source ordering is irrelevant).
                src_ap = m[:].unsqueeze(1).broadcast_to([128, att_reps, MTILE])
                r0 = g * H * NMAX
                nc.sync.dma_start(ya[r0 : r0 + H * NMAX, :], src_ap)

    nc.compile()
    return nc


def make_in_maps(x, batch_ids):
    x = np.ascontiguousarray(np.asarray(x), dtype=np.float32)
    bids = np.asarray(batch_ids).astype(np.int64)
    counts = np.bincount(bids, minlength=B)[:B]
    starts = np.cumsum(counts) - counts

    in_maps = []
    p = np.arange(128)
    for k in range(NCORES):
        g0 = k * GPC
        r0 = int(starts[g0])
        r1 = int(starts[g0 + GPC - 1] + counts[g0 + GPC - 1])
        xs = np.zeros((XS_ROWS, F), np.float32)
        xs[: r1 - r0] = x[r0:r1]
        rel = (starts[g0 : g0 + GPC] - r0).astype(np.int64)
        cnt = counts[g0 : g0 + GPC].astype(np.int64)

        # per-graph gather base rows: partition p reads xs rows rel_g+6p..+5
        gidx = (rel[None, :] + 6 * p[:, None]).astype(np.int32)   # [128, GPC]

        meta = np.zeros((128, 64), np.float32)
        meta[:, :GPC] = cnt.astype(np.float32)           # replicated counts
        meta[:GPC, 8] = cnt.astype(np.float32)           # per-partition counts
        meta[:, META_GIDX0 : META_GIDX0 + GPC] = gidx.view(np.float32)

        in_maps.append({"xs": xs, "meta": meta})
    return in_maps


def assemble(results):
    dense = np.concatenate(
        [r["yd"].reshape(GPC, NMAX, F) for r in results], axis=0
    )
    dmask = np.concatenate([r["ym"] for r in results], axis=0).astype(bool)
    attn = np.concatenate(
        [r["ya"].reshape(GPC, H, NMAX, NMAX) for r in results], axis=0
    )
    return dense, dmask, attn


def kernel(x, batch_ids, batch_size, max_num_nodes, num_heads, **_):
    from concourse.bass_utils import run_bass_kernel_spmd

    in_maps = make_in_maps(x, batch_ids)
    nc = build_nc()
    res = run_bass_kernel_spmd(nc, in_maps, core_ids=list(range(NCORES)))
    return assemble(res.results)
